# revision 1
# baseline (speedup 1.0000x reference)
"""CP-decomposed conv (pointwise -> depthwise-h -> depthwise-w -> pointwise)
as a Bass/Tile kernel on 8 TRN2 NeuronCores.

Strategy:
  - Data-parallel over batch: 32 images -> 4 per core, no collectives.
  - Fold the depthwise h-conv into the first pointwise conv:
      y2[r,i,w] = sum_{h,c} (factor3[c,r]*factor1[h,r]) * x[c,i+h,w]
    -> 6 accumulating matmuls per PSUM tile (3 h-shifts x 2 C-chunks).
  - Fold the depthwise w-conv into the final projection:
      out[f,i,j] = sum_{w,r} (factor0[f,r]*factor2[w,r]) * y2[r,i,j+w]
    -> 3 accumulating matmuls per PSUM tile (w-shifted rhs APs).
  - All matmuls in float32r (full PE rate at N>=256, ~1e-4 rel err).
  - DVE/ACT only do PSUM->SBUF copies; DMA is the roofline (~110MB/core).
"""

import sys
import numpy as np

for _p in ("/opt/trn_rl_repo",):
    if _p not in sys.path:
        sys.path.insert(0, _p)

B, C, H, W = 32, 256, 96, 96
F, FH, FW, R = 512, 3, 3, 128
OH, OW = H - FH + 1, W - FW + 1  # 94, 94
NCORES = 8
BLOC = B // NCORES  # 4 images per core

# output-row strips and row-tiles within a strip (all row-tiles >= 3 rows so
# every matmul free dim >= 256 -> full float32r rate)
STRIPS = [(0, 47), (47, 47)]
ROW_TILES = [5] * 8 + [4, 3]  # sums to 47

_NC_CACHE = {}


def _build_nc():
    import concourse.bacc as bacc
    import concourse.mybir as mybir
    import concourse.tile as tile

    f32 = mybir.dt.float32
    f32r = mybir.dt.float32r

    nc = bacc.Bacc("TRN2", target_bir_lowering=False, debug=True)

    xd = nc.dram_tensor("x", [BLOC, C, H, W], f32, kind="ExternalInput")
    wad = nc.dram_tensor("wa", [FH, 2, 128, R], f32, kind="ExternalInput")
    wbd = nc.dram_tensor("wb", [FW, 4, R, 128], f32, kind="ExternalInput")
    od = nc.dram_tensor("out", [BLOC, F, OH, OW], f32, kind="ExternalOutput")

    with tile.TileContext(nc) as tc:
        with (
            tc.tile_pool(name="wpool", bufs=1) as wpool,
            tc.tile_pool(name="xs", bufs=2) as xs_pool,
            tc.tile_pool(name="y2", bufs=2) as y2_pool,
            tc.tile_pool(name="osb", bufs=2) as osb_pool,
            tc.tile_pool(name="psA", bufs=3, space="PSUM") as psA,
            tc.tile_pool(name="psD", bufs=4, space="PSUM") as psD,
        ):
            # stationary weights, rounded to f32r at DMA time via bitcast
            wa_sb = wpool.tile([128, FH * 2, R], f32r)
            for h in range(FH):
                for ch in range(2):
                    nc.sync.dma_start(
                        wa_sb[:, h * 2 + ch, :], wad[h, ch].bitcast(f32r)
                    )
            wb_sb = wpool.tile([128, FW * 4, 128], f32r)
            for w in range(FW):
                for fc in range(4):
                    nc.sync.dma_start(
                        wb_sb[:, w * 4 + fc, :], wbd[w, fc].bitcast(f32r)
                    )

            copy_i = 0  # alternate PSUM->SBUF copies between DVE and ACT

            def psum_copy(dst, src):
                nonlocal copy_i
                if copy_i % 5 < 3:
                    nc.vector.tensor_copy(dst, src)
                else:
                    nc.scalar.copy(dst, src)
                copy_i += 1

            for b in range(BLOC):
                for i0, S in STRIPS:
                    nrows_in = S + 2  # x halo
                    xs_t = xs_pool.tile([128, 2, nrows_in * W], f32r)
                    for ch in range(2):
                        nc.sync.dma_start(
                            xs_t[:, ch, :],
                            xd[b, ch * 128 : (ch + 1) * 128, i0 : i0 + nrows_in, :]
                            .bitcast(f32r),
                        )

                    y2_t = y2_pool.tile([128, S, W], f32r)

                    # stage A+B: pointwise C->R with h-conv folded in
                    r0 = 0
                    for nr in ROW_TILES:
                        pa = psA.tile([128, nr, W], f32)
                        k = 0
                        for h in range(FH):
                            for ch in range(2):
                                nc.tensor.matmul(
                                    pa[:],
                                    wa_sb[:, h * 2 + ch, :],
                                    xs_t[:, ch, (r0 + h) * W : (r0 + h + nr) * W],
                                    start=(k == 0),
                                    stop=(k == 5),
                                )
                                k += 1
                        psum_copy(y2_t[:, r0 : r0 + nr, :], pa[:])
                        r0 += nr

                    # stage C+D: projection R->F with w-conv folded in
                    for fc in range(4):
                        ot = osb_pool.tile([128, S, OW], f32)
                        r0 = 0
                        for nr in ROW_TILES:
                            pd = psD.tile([128, nr, OW], f32)
                            for w in range(FW):
                                nc.tensor.matmul(
                                    pd[:],
                                    wb_sb[:, w * 4 + fc, :],
                                    y2_t[:, r0 : r0 + nr, w : w + OW],
                                    start=(w == 0),
                                    stop=(w == FW - 1),
                                )
                            psum_copy(ot[:, r0 : r0 + nr, :], pd[:])
                            r0 += nr
                        nc.sync.dma_start(
                            od[b, fc * 128 : (fc + 1) * 128, i0 : i0 + S, :],
                            ot[:],
                        )

    nc.compile()
    return nc


def _get_nc():
    if "nc" not in _NC_CACHE:
        _NC_CACHE["nc"] = _build_nc()
    return _NC_CACHE["nc"]


def kernel(x, factor0, factor1, factor2, factor3):
    from concourse import bass_utils

    x = np.ascontiguousarray(x, dtype=np.float32)
    factor0 = np.asarray(factor0, dtype=np.float32)
    factor1 = np.asarray(factor1, dtype=np.float32)
    factor2 = np.asarray(factor2, dtype=np.float32)
    factor3 = np.asarray(factor3, dtype=np.float32)

    # wa[h, ch, c', r] = factor3[ch*128+c', r] * factor1[h, r]
    wa = (factor3[None, :, :] * factor1[:, None, :]).reshape(FH, 2, 128, R)
    wa = np.ascontiguousarray(wa, dtype=np.float32)
    # wb[w, fc, r, f'] = factor0[fc*128+f', r] * factor2[w, r]
    g = factor0[None, :, :] * factor2[:, None, :]  # [FW, F, R]
    wb = np.ascontiguousarray(
        g.reshape(FW, 4, 128, R).transpose(0, 1, 3, 2), dtype=np.float32
    )

    nc = _get_nc()
    in_maps = [
        {"x": x[c * BLOC : (c + 1) * BLOC], "wa": wa, "wb": wb}
        for c in range(NCORES)
    ]
    res = bass_utils.run_bass_kernel_spmd(nc, in_maps, list(range(NCORES)))
    return np.concatenate([res.results[c]["out"] for c in range(NCORES)], axis=0)


# revision 3
# speedup vs baseline: 1.1810x; 1.1810x over previous
"""CP-decomposed conv (pointwise -> depthwise-h -> depthwise-w -> pointwise)
as a Bass/Tile kernel on 8 TRN2 NeuronCores.

Strategy:
  - Data-parallel over batch: 32 images -> 4 per core, no collectives.
  - Fold the depthwise h-conv into the first pointwise conv:
      y2[r,i,w] = sum_{h,c} (factor3[c,r]*factor1[h,r]) * x[c,i+h,w]
    -> 6 accumulating matmuls per PSUM tile (3 h-shifts x 2 C-chunks).
  - Fold the depthwise w-conv into the final projection:
      out[f,i,j] = sum_{w,r} (factor0[f,r]*factor2[w,r]) * y2[r,i,j+w]
    -> 3 accumulating matmuls per PSUM tile (w-shifted rhs APs).
  - All matmuls in float32r (full PE rate at N>=256, ~1e-4 rel err).
  - DVE/ACT only do PSUM->SBUF copies; DMA is the roofline (~110MB/core).
"""

import sys
import numpy as np

for _p in ("/opt/trn_rl_repo",):
    if _p not in sys.path:
        sys.path.insert(0, _p)

B, C, H, W = 32, 256, 96, 96
F, FH, FW, R = 512, 3, 3, 128
OH, OW = H - FH + 1, W - FW + 1  # 94, 94
NCORES = 8
BLOC = B // NCORES  # 4 images per core

# output-row strips and row-tiles within a strip (all row-tiles >= 3 rows so
# every matmul free dim >= 256 -> full float32r rate)
STRIPS = [(0, 47), (47, 47)]
ROW_TILES = [5] * 8 + [4, 3]  # sums to 47

_NC_CACHE = {}


def _build_nc():
    import concourse.bacc as bacc
    import concourse.mybir as mybir
    import concourse.tile as tile

    f32 = mybir.dt.float32
    f32r = mybir.dt.float32r

    nc = bacc.Bacc("TRN2", target_bir_lowering=False, debug=True)

    xd = nc.dram_tensor("x", [BLOC, C, H, W], f32, kind="ExternalInput")
    wad = nc.dram_tensor("wa", [FH, 2, 128, R], f32, kind="ExternalInput")
    wbd = nc.dram_tensor("wb", [FW, 4, R, 128], f32, kind="ExternalInput")
    od = nc.dram_tensor("out", [BLOC, F, OH, OW], f32, kind="ExternalOutput")

    with tile.TileContext(nc) as tc:
        with (
            tc.tile_pool(name="wpool", bufs=1) as wpool,
            tc.tile_pool(name="xs", bufs=3) as xs_pool,
            tc.tile_pool(name="y2", bufs=2) as y2_pool,
            tc.tile_pool(name="osb", bufs=3) as osb_pool,
            tc.tile_pool(name="psA", bufs=4, space="PSUM") as psA,
            tc.tile_pool(name="psD", bufs=4, space="PSUM") as psD,
        ):
            # stationary weights, rounded to f32r at DMA time via bitcast
            wa_sb = wpool.tile([128, FH * 2, R], f32r)
            for h in range(FH):
                for ch in range(2):
                    nc.sync.dma_start(
                        wa_sb[:, h * 2 + ch, :], wad[h, ch].bitcast(f32r)
                    )
            wb_sb = wpool.tile([128, FW * 4, 128], f32r)
            for w in range(FW):
                for fc in range(4):
                    nc.sync.dma_start(
                        wb_sb[:, w * 4 + fc, :], wbd[w, fc].bitcast(f32r)
                    )

            copy_i = 0  # alternate PSUM->SBUF copies between DVE and ACT

            def psum_copy(dst, src):
                nonlocal copy_i
                if copy_i % 5 < 3:
                    nc.vector.tensor_copy(dst, src)
                else:
                    nc.scalar.copy(dst, src)
                copy_i += 1

            for b in range(BLOC):
                for i0, S in STRIPS:
                    nrows_in = S + 2  # x halo
                    xs_t = xs_pool.tile([128, 2, nrows_in * W], f32r)
                    for ch in range(2):
                        # input loads on the ACT HWDGE ring; output stores on
                        # the SP ring -> reads and writes overlap per engine
                        nc.scalar.dma_start(
                            xs_t[:, ch, :],
                            xd[b, ch * 128 : (ch + 1) * 128, i0 : i0 + nrows_in, :]
                            .bitcast(f32r),
                        )

                    y2_t = y2_pool.tile([128, S, W], f32r)

                    # stage A+B: pointwise C->R with h-conv folded in
                    r0 = 0
                    for nr in ROW_TILES:
                        pa = psA.tile([128, nr, W], f32)
                        k = 0
                        for h in range(FH):
                            for ch in range(2):
                                nc.tensor.matmul(
                                    pa[:],
                                    wa_sb[:, h * 2 + ch, :],
                                    xs_t[:, ch, (r0 + h) * W : (r0 + h + nr) * W],
                                    start=(k == 0),
                                    stop=(k == 5),
                                )
                                k += 1
                        psum_copy(y2_t[:, r0 : r0 + nr, :], pa[:])
                        r0 += nr

                    # stage C+D: projection R->F with w-conv folded in
                    for fc in range(4):
                        ot = osb_pool.tile([128, S, OW], f32)
                        r0 = 0
                        for nr in ROW_TILES:
                            pd = psD.tile([128, nr, OW], f32)
                            for w in range(FW):
                                nc.tensor.matmul(
                                    pd[:],
                                    wb_sb[:, w * 4 + fc, :],
                                    y2_t[:, r0 : r0 + nr, w : w + OW],
                                    start=(w == 0),
                                    stop=(w == FW - 1),
                                )
                            psum_copy(ot[:, r0 : r0 + nr, :], pd[:])
                            r0 += nr
                        nc.sync.dma_start(
                            od[b, fc * 128 : (fc + 1) * 128, i0 : i0 + S, :],
                            ot[:],
                        )

    nc.compile()
    return nc


def _get_nc():
    if "nc" not in _NC_CACHE:
        _NC_CACHE["nc"] = _build_nc()
    return _NC_CACHE["nc"]


def kernel(x, factor0, factor1, factor2, factor3):
    from concourse import bass_utils

    x = np.ascontiguousarray(x, dtype=np.float32)
    factor0 = np.asarray(factor0, dtype=np.float32)
    factor1 = np.asarray(factor1, dtype=np.float32)
    factor2 = np.asarray(factor2, dtype=np.float32)
    factor3 = np.asarray(factor3, dtype=np.float32)

    # wa[h, ch, c', r] = factor3[ch*128+c', r] * factor1[h, r]
    wa = (factor3[None, :, :] * factor1[:, None, :]).reshape(FH, 2, 128, R)
    wa = np.ascontiguousarray(wa, dtype=np.float32)
    # wb[w, fc, r, f'] = factor0[fc*128+f', r] * factor2[w, r]
    g = factor0[None, :, :] * factor2[:, None, :]  # [FW, F, R]
    wb = np.ascontiguousarray(
        g.reshape(FW, 4, 128, R).transpose(0, 1, 3, 2), dtype=np.float32
    )

    nc = _get_nc()
    in_maps = [
        {"x": x[c * BLOC : (c + 1) * BLOC], "wa": wa, "wb": wb}
        for c in range(NCORES)
    ]
    res = bass_utils.run_bass_kernel_spmd(nc, in_maps, list(range(NCORES)))
    return np.concatenate([res.results[c]["out"] for c in range(NCORES)], axis=0)


# revision 4
# speedup vs baseline: 1.2926x; 1.0946x over previous
"""CP-decomposed conv (pointwise -> depthwise-h -> depthwise-w -> pointwise)
as a Bass/Tile kernel on 8 TRN2 NeuronCores.

Strategy:
  - Data-parallel over batch: 32 images -> 4 per core, no collectives.
  - Fold the depthwise h-conv into the first pointwise conv:
      y2[r,i,w] = sum_{h,c} (factor3[c,r]*factor1[h,r]) * x[c,i+h,w]
    -> 6 accumulating matmuls per PSUM tile (3 h-shifts x 2 C-chunks).
    y2 stays in PSUM.
  - Depthwise w-conv straight out of PSUM on ACT+DVE with per-partition
    scalars (factor2[w,r] lives on partition r):
      y3 = sum_w y2[:,:,w:w+94] * f2[w]   (1 ACT copy-scale + 2 DVE STT)
  - Final projection R->F: one matmul per (fc, row-tile).
  - All matmuls in float32r (full PE rate at N>=256, ~1e-4 rel err).
  - Input DMAs ride the ACT HWDGE ring, output DMAs the SP ring.
"""

import sys
import numpy as np

for _p in ("/opt/trn_rl_repo",):
    if _p not in sys.path:
        sys.path.insert(0, _p)

B, C, H, W = 32, 256, 96, 96
F, FH, FW, R = 512, 3, 3, 128
OH, OW = H - FH + 1, W - FW + 1  # 94, 94
NCORES = 8
BLOC = B // NCORES  # 4 images per core

# output-row strips and row-tiles within a strip (all row-tiles >= 3 rows so
# every matmul free dim >= 256 -> full float32r rate)
STRIPS = [(0, 47), (47, 47)]
ROW_TILES = [5] * 8 + [4, 3]  # sums to 47

_NC_CACHE = {}


def _build_nc():
    import concourse.bacc as bacc
    import concourse.mybir as mybir
    import concourse.tile as tile

    f32 = mybir.dt.float32
    f32r = mybir.dt.float32r
    mult = mybir.AluOpType.mult
    add = mybir.AluOpType.add

    nc = bacc.Bacc("TRN2", target_bir_lowering=False, debug=True)

    xd = nc.dram_tensor("x", [BLOC, C, H, W], f32, kind="ExternalInput")
    wad = nc.dram_tensor("wa", [FH, 2, 128, R], f32, kind="ExternalInput")
    wbd = nc.dram_tensor("wb", [4, R, 128], f32, kind="ExternalInput")
    wcd = nc.dram_tensor("wc", [R, FW], f32, kind="ExternalInput")
    od = nc.dram_tensor("out", [BLOC, F, OH, OW], f32, kind="ExternalOutput")

    with tile.TileContext(nc) as tc:
        with (
            tc.tile_pool(name="wpool", bufs=1) as wpool,
            tc.tile_pool(name="xs", bufs=2) as xs_pool,
            tc.tile_pool(name="y3", bufs=2) as y3_pool,
            tc.tile_pool(name="osb", bufs=3) as osb_pool,
            tc.tile_pool(name="psA", bufs=4, space="PSUM") as psA,
            tc.tile_pool(name="psD", bufs=4, space="PSUM") as psD,
        ):
            # stationary weights, rounded to f32r at DMA time via bitcast
            wa_sb = wpool.tile([128, FH * 2, R], f32r)
            for h in range(FH):
                for ch in range(2):
                    nc.sync.dma_start(
                        wa_sb[:, h * 2 + ch, :], wad[h, ch].bitcast(f32r)
                    )
            wb_sb = wpool.tile([128, 4, 128], f32r)
            for fc in range(4):
                nc.sync.dma_start(wb_sb[:, fc, :], wbd[fc].bitcast(f32r))
            wc_sb = wpool.tile([128, FW], f32)
            nc.sync.dma_start(wc_sb[:], wcd[:])

            copy_i = 0  # alternate stage-D PSUM->SBUF copies DVE/ACT

            def psum_copy(dst, src):
                nonlocal copy_i
                if copy_i % 2 == 0:
                    nc.vector.tensor_copy(dst, src)
                else:
                    nc.scalar.copy(dst, src)
                copy_i += 1

            for b in range(BLOC):
                for i0, S in STRIPS:
                    nrows_in = S + 2  # x halo
                    xs_t = xs_pool.tile([128, 2, nrows_in * W], f32r)
                    for ch in range(2):
                        # input loads on the ACT HWDGE ring; output stores on
                        # the SP ring -> reads and writes overlap per engine
                        nc.scalar.dma_start(
                            xs_t[:, ch, :],
                            xd[b, ch * 128 : (ch + 1) * 128, i0 : i0 + nrows_in, :]
                            .bitcast(f32r),
                        )

                    y3_t = y3_pool.tile([128, S, OW], f32r)

                    # stage A+B: pointwise C->R with h-conv folded in (PSUM),
                    # then stage C: w-conv PSUM->SBUF via per-partition scalars
                    r0 = 0
                    for nr in ROW_TILES:
                        pa = psA.tile([128, nr, W], f32)
                        k = 0
                        for h in range(FH):
                            for ch in range(2):
                                nc.tensor.matmul(
                                    pa[:],
                                    wa_sb[:, h * 2 + ch, :],
                                    xs_t[:, ch, (r0 + h) * W : (r0 + h + nr) * W],
                                    start=(k == 0),
                                    stop=(k == 5),
                                )
                                k += 1
                        dst = y3_t[:, r0 : r0 + nr, :]
                        nc.scalar.mul(dst, pa[:, :, 0:OW], wc_sb[:, 0:1])
                        nc.vector.scalar_tensor_tensor(
                            dst, pa[:, :, 1 : 1 + OW], wc_sb[:, 1:2], dst,
                            op0=mult, op1=add,
                        )
                        nc.vector.scalar_tensor_tensor(
                            dst, pa[:, :, 2 : 2 + OW], wc_sb[:, 2:3], dst,
                            op0=mult, op1=add,
                        )
                        r0 += nr

                    # stage D: projection R->F, one matmul per (fc, row-tile)
                    for fc in range(4):
                        ot = osb_pool.tile([128, S, OW], f32)
                        r0 = 0
                        for nr in ROW_TILES:
                            pd = psD.tile([128, nr, OW], f32)
                            nc.tensor.matmul(
                                pd[:],
                                wb_sb[:, fc, :],
                                y3_t[:, r0 : r0 + nr, :],
                                start=True,
                                stop=True,
                            )
                            psum_copy(ot[:, r0 : r0 + nr, :], pd[:])
                            r0 += nr
                        nc.sync.dma_start(
                            od[b, fc * 128 : (fc + 1) * 128, i0 : i0 + S, :],
                            ot[:],
                        )

    nc.compile()
    return nc


def _get_nc():
    if "nc" not in _NC_CACHE:
        _NC_CACHE["nc"] = _build_nc()
    return _NC_CACHE["nc"]


def _prep_weights(factor0, factor1, factor2, factor3):
    # wa[h, ch, c', r] = factor3[ch*128+c', r] * factor1[h, r]
    wa = (factor3[None, :, :] * factor1[:, None, :]).reshape(FH, 2, 128, R)
    wa = np.ascontiguousarray(wa, dtype=np.float32)
    # wb[fc, r, f'] = factor0[fc*128+f', r]
    wb = np.ascontiguousarray(
        factor0.reshape(4, 128, R).transpose(0, 2, 1), dtype=np.float32
    )
    # wc[r, w] = factor2[w, r]
    wc = np.ascontiguousarray(factor2.T, dtype=np.float32)
    return wa, wb, wc


def kernel(x, factor0, factor1, factor2, factor3):
    from concourse import bass_utils

    x = np.ascontiguousarray(x, dtype=np.float32)
    factor0 = np.asarray(factor0, dtype=np.float32)
    factor1 = np.asarray(factor1, dtype=np.float32)
    factor2 = np.asarray(factor2, dtype=np.float32)
    factor3 = np.asarray(factor3, dtype=np.float32)

    wa, wb, wc = _prep_weights(factor0, factor1, factor2, factor3)

    nc = _get_nc()
    in_maps = [
        {"x": x[c * BLOC : (c + 1) * BLOC], "wa": wa, "wb": wb, "wc": wc}
        for c in range(NCORES)
    ]
    res = bass_utils.run_bass_kernel_spmd(nc, in_maps, list(range(NCORES)))
    return np.concatenate([res.results[c]["out"] for c in range(NCORES)], axis=0)
